# revision 22
# baseline (speedup 1.0000x reference)
"""Trainium2 Bass kernel for GraphTransformerNet (star-graph TransformerConv).

Shapes (hardcoded): B=1024 graphs, N=128 neighbors, D=256 in-dim,
H=4 heads x C=64 = F=256 out-dim. Data-parallel over 8 NeuronCores
(128 graphs/core).

Key structure (v2 — scores via host-folded q):
  The attention logits only need q.k = x @ (Wk q) + e @ (We q), so the
  per-graph q vector is folded into tiny per-graph weight columns
  wkq[d,g,h] host-side. That removes the whole kT projection pipeline
  (a third of PE streaming) and its PSUM->SBUF casts. Per graph the PE
  runs 4 big matmuls ([v|skip] from x, v from e; stationary = the
  graph's x/e d-chunk) plus 4 nearly-free 4-column score matmuls that
  reuse those stationaries.

  Softmax is max-free (scores ~ N(0,2), no overflow): Act exps the
  [128n, 32(g,h)] score block; the sums over n come from a ones-vector
  matmul; normalization happens on the host (agg and sums ship raw).
  Aggregation = 4 ap=1 matmuls per graph into a persistent PSUM tile
  [128 f_loc, 2 fc, BG], consumed one group behind the producer so the
  PE never waits on Act. Central skip projection runs once at the end
  into its own PSUM; host adds agg/sums to it in f32.

  Inputs ship as [D, BG, x|e] so each group needs just two 2D DMAs;
  skip rows buffer per-group and leave in one DMA as bf16 to an
  [N, BG, F] layout (host transposes back).
"""

import sys

import numpy as np

for _p in ("/opt/trn_rl_repo",):
    if _p not in sys.path:
        sys.path.insert(0, _p)

import ml_dtypes

import concourse.bacc as bacc
import concourse.bass as bass
import concourse.mybir as mybir
from concourse.bass import MemorySpace
from concourse.tile import TileContext

BF16 = mybir.dt.bfloat16
F32 = mybir.dt.float32
AFT = mybir.ActivationFunctionType

B, N, D, H, C = 1024, 128, 256, 4, 64
F = H * C            # 256
NCORES = 8
BG = B // NCORES     # 128 graphs per core
GROUP = 8            # graphs per group (softmax/DMA batch)
NG = BG // GROUP     # 16 groups
ROWS = N + 1         # 129 output rows per graph

_cached = {}


def _build_nc():
    nc = bacc.Bacc()

    xe_d = nc.dram_tensor("xe", [D, BG, 2 * N], BF16, kind="ExternalInput")
    wkq_d = nc.dram_tensor("wkq", [D, BG, H], BF16, kind="ExternalInput")
    weq_d = nc.dram_tensor("weq", [D, BG, H], BF16, kind="ExternalInput")
    wvs_d = nc.dram_tensor("wvs", [D, 2 * F], BF16, kind="ExternalInput")
    we_d = nc.dram_tensor("we", [D, F], BF16, kind="ExternalInput")
    ct_d = nc.dram_tensor("ct", [D, BG], BF16, kind="ExternalInput")
    ones_d = nc.dram_tensor("ones", [128, 1], BF16, kind="ExternalInput")

    skip_d = nc.dram_tensor("skip", [N, BG, F], BF16, kind="ExternalOutput")
    aggT_d = nc.dram_tensor("aggT", [128, 2, BG], F32, kind="ExternalOutput")
    skT_d = nc.dram_tensor("skT", [128, 2, BG], F32, kind="ExternalOutput")
    sums_d = nc.dram_tensor("sums", [1, BG * H], F32, kind="ExternalOutput")

    with TileContext(nc) as tc:
        with (
            tc.tile_pool(name="consts", bufs=1) as consts,
            tc.tile_pool(name="io", bufs=8) as io,
            tc.tile_pool(name="vsb", bufs=2 * GROUP + 2) as v_pool,
            tc.tile_pool(name="skipsb", bufs=2) as skip_pool,
            tc.tile_pool(name="expsb", bufs=3) as exp_pool,
            tc.tile_pool(name="scsum", bufs=2) as scsum_pool,
            tc.tile_pool(name="misc", bufs=6) as misc,
            tc.tile_pool(name="vs_ps", bufs=3, space=MemorySpace.PSUM) as vs_psp,
            tc.tile_pool(name="sc_ps", bufs=2, space=MemorySpace.PSUM) as sc_psp,
            tc.tile_pool(name="agg_ps", bufs=1, space=MemorySpace.PSUM) as agg_psp,
            tc.tile_pool(name="sum_ps", bufs=1, space=MemorySpace.PSUM) as sum_psp,
            tc.tile_pool(name="skc_ps", bufs=1, space=MemorySpace.PSUM) as skc_psp,
        ):
            # ---- constants (spread across trigger queues for a fast ramp) ----
            wvs_sb, we_sb, wkq_sb, weq_sb, ct_sb = [], [], [], [], []
            for dc in range(2):
                dsl = slice(dc * 128, (dc + 1) * 128)
                t = consts.tile([128, 2 * F], BF16, tag=f"wvs{dc}")
                nc.sync.dma_start(t[:, :], wvs_d[dsl, :]); wvs_sb.append(t)
                t = consts.tile([128, F], BF16, tag=f"we{dc}")
                nc.sync.dma_start(t[:, :], we_d[dsl, :]); we_sb.append(t)
                t = consts.tile([128, BG, H], BF16, tag=f"wkq{dc}")
                nc.gpsimd.dma_start(t[:, :, :], wkq_d[dsl, :, :]); wkq_sb.append(t)
                t = consts.tile([128, BG, H], BF16, tag=f"weq{dc}")
                nc.scalar.dma_start(t[:, :, :], weq_d[dsl, :, :]); weq_sb.append(t)
                t = consts.tile([128, BG], BF16, tag=f"ct{dc}")
                nc.scalar.dma_start(t[:, :], ct_d[dsl, :]); ct_sb.append(t)
            ones_sb = consts.tile([128, 1], BF16, tag="ones")
            nc.gpsimd.dma_start(ones_sb[:, :], ones_d[:, :])

            # persistent PSUM: unnormalized aggregated messages in
            # [f_loc, fc, g] layout, and per-(g,h) exp-sums. Each element
            # is written by exactly one matmul; start=True only on the
            # very first write into each bank (clears has_written for the
            # whole bank -> first touch of every address overwrites).
            agg_ps = agg_psp.tile([128, 2, BG], F32, tag="agg")
            sums_ps = sum_psp.tile([1, BG * H], F32, tag="sums")

            # central skip projection runs up front (only needs consts) so
            # the tail after the last group stays short
            skc_ps = skc_psp.tile([128, 2, BG], F32, tag="skc")
            for fc in range(2):
                fsl = slice(F + fc * 128, F + (fc + 1) * 128)
                for dc in range(2):
                    nc.tensor.matmul(skc_ps[:, fc, :], wvs_sb[dc][:, fsl],
                                     ct_sb[dc][:, :],
                                     start=(fc == 0 and dc == 0),
                                     stop=(fc == 1 and dc == 1),
                                     skip_group_check=(fc == 1))
            skT_sb = misc.tile([128, 2, BG], F32, tag="skT")
            nc.scalar.activation(skT_sb[:, :, :], skc_ps[:, :, :], AFT.Copy)
            nc.sync.dma_start(skT_d[:, :, :], skT_sb[:, :, :])

            state = {}

            def consume(j):
                exp_sb, v_sbs = state.pop(j)
                nc.tensor.matmul(sums_ps[0:1, j * 32:(j + 1) * 32],
                                 ones_sb[:, :], exp_sb[:, :],
                                 start=(j == 0), stop=(j == NG - 1),
                                 skip_group_check=(j > 0))
                for gg in range(GROUP):
                    g = j * GROUP + gg
                    for fc in range(2):
                        for hh in range(2):
                            h = fc * 2 + hh
                            first = (g == 0 and fc == 0 and hh == 0)
                            last = (g == BG - 1 and fc == 1 and hh == 1)
                            nc.tensor.matmul(
                                agg_ps[hh * 64:(hh + 1) * 64, fc, g:g + 1],
                                v_sbs[gg][:, fc * 128 + hh * 64:
                                          fc * 128 + (hh + 1) * 64],
                                exp_sb[:, gg * 4 + h:gg * 4 + h + 1],
                                start=first, stop=last,
                                skip_group_check=not first)

            # ---- main loop over groups of 8 graphs ----
            for grp in range(NG):
                g0 = grp * GROUP
                xe = []
                for dc in range(2):
                    dsl = slice(dc * 128, (dc + 1) * 128)
                    t = io.tile([128, GROUP, 2 * N], BF16, tag=f"xe{dc}")
                    if grp == 0:
                        # first tiles gate the PE ramp: split each across two
                        # trigger queues so the four halves transfer at once
                        h = GROUP // 2
                        eng0 = nc.gpsimd if dc == 0 else nc.gpsimd
                        eng1 = nc.sync if dc == 0 else nc.scalar
                        eng0.dma_start(t[:, 0:h, :], xe_d[dsl, g0:g0 + h, :])
                        eng1.dma_start(t[:, h:GROUP, :],
                                       xe_d[dsl, g0 + h:g0 + GROUP, :])
                    else:
                        # two queues so back-to-back tiles transfer in
                        # parallel while the prefetch pipeline fills
                        eng = nc.gpsimd if dc == 0 else nc.sync
                        eng.dma_start(t[:, :, :],
                                      xe_d[dsl, g0:g0 + GROUP, :])
                    xe.append(t)

                # four score regions, one per contraction chunk, every write
                # an overwrite: a tiny accumulating matmul pays the PSUM
                # read-modify-write latency (~150-290ns) that can't pipeline
                # over 4 addresses, while tiny overwrites cost ~19ns. The
                # regions are summed on DVE instead.
                sc_ps = sc_psp.tile([128, 4, GROUP * H], F32, tag="scps")
                skip_t = skip_pool.tile([128, GROUP, F], BF16, tag="skipsb")
                v_sbs = []

                for gg in range(GROUP):
                    g = g0 + gg
                    ssl = slice(gg * 4, gg * 4 + 4)
                    xs = [xe[0][:, gg, 0:N], xe[1][:, gg, 0:N]]
                    es = [xe[0][:, gg, N:2 * N], xe[1][:, gg, N:2 * N]]
                    vs_ps = vs_psp.tile([128, 2 * F], F32, tag="vsps")
                    nc.tensor.matmul(vs_ps[:, :], xs[0], wvs_sb[0][:, :],
                                     start=True, stop=False)
                    nc.tensor.matmul(sc_ps[:, 0, ssl], xs[0],
                                     wkq_sb[0][:, g, :],
                                     start=(gg == 0), stop=False,
                                     skip_group_check=(gg > 0))
                    nc.tensor.matmul(vs_ps[:, :], xs[1], wvs_sb[1][:, :],
                                     start=False, stop=False)
                    nc.tensor.matmul(sc_ps[:, 1, ssl], xs[1],
                                     wkq_sb[1][:, g, :],
                                     start=False, stop=False,
                                     skip_group_check=True)
                    nc.tensor.matmul(vs_ps[:, 0:F], es[0], we_sb[0][:, :],
                                     start=False, stop=False,
                                     skip_group_check=True)
                    nc.tensor.matmul(sc_ps[:, 2, ssl], es[0],
                                     weq_sb[0][:, g, :],
                                     start=False, stop=False,
                                     skip_group_check=True)
                    nc.tensor.matmul(vs_ps[:, 0:F], es[1], we_sb[1][:, :],
                                     start=False, stop=True,
                                     skip_group_check=True)
                    nc.tensor.matmul(sc_ps[:, 3, ssl], es[1],
                                     weq_sb[1][:, g, :],
                                     start=False, stop=(gg == GROUP - 1),
                                     skip_group_check=True)

                    v_sb = v_pool.tile([128, F], BF16, tag="vsb")
                    nc.vector.tensor_copy(v_sb[:, :], vs_ps[:, 0:F])
                    v_sbs.append(v_sb)
                    nc.scalar.activation(skip_t[:, gg, :], vs_ps[:, F:2 * F],
                                         AFT.Copy)
                nc.sync.dma_start(skip_d[:, g0:g0 + GROUP, :], skip_t[:, :, :])

                # combine the four regions on DVE (at most one PSUM operand
                # per op), then a single exp on Act
                sc_t = scsum_pool.tile([128, 3, GROUP * H], F32, tag="scsum")
                nc.vector.tensor_copy(sc_t[:, 0, :], sc_ps[:, 1, :])
                nc.vector.tensor_add(sc_t[:, 1, :], sc_ps[:, 0, :],
                                     sc_t[:, 0, :])
                nc.vector.tensor_add(sc_t[:, 0, :], sc_ps[:, 2, :],
                                     sc_t[:, 1, :])
                nc.vector.tensor_add(sc_t[:, 2, :], sc_ps[:, 3, :],
                                     sc_t[:, 0, :])
                exp_sb = exp_pool.tile([128, GROUP * H], BF16, tag="expsb")
                nc.scalar.activation(exp_sb[:, :], sc_t[:, 2, :], AFT.Exp)
                state[grp] = (exp_sb, v_sbs)
                if grp >= 1:
                    consume(grp - 1)

            consume(NG - 1)

            # ---- ship raw agg / sums; host normalizes ----
            aggT_sb = misc.tile([128, 2, BG], F32, tag="aggT")
            nc.vector.tensor_copy(aggT_sb[:, :, :], agg_ps[:, :, :])
            nc.sync.dma_start(aggT_d[:, :, :], aggT_sb[:, :, :])
            sums_sb = misc.tile([1, BG * H], F32, tag="sumsb")
            nc.vector.tensor_copy(sums_sb[:, :], sums_ps[:, :])
            nc.sync.dma_start(sums_d[:, :], sums_sb[:, :])

    nc.compile()
    return nc


def kernel(**inputs):
    x = np.asarray(inputs["neighbor_node_features"], dtype=np.float32)   # [B, N, D]
    e = np.asarray(inputs["edge_features"], dtype=np.float32)            # [B, N, D]
    cen = np.asarray(inputs["central_node_features"], dtype=np.float32)  # [B, 1, D]
    Wq = np.asarray(inputs["Wq"], dtype=np.float32)
    Wk = np.asarray(inputs["Wk"], dtype=np.float32)
    Wv = np.asarray(inputs["Wv"], dtype=np.float32)
    We = np.asarray(inputs["We"], dtype=np.float32)
    Ws = np.asarray(inputs["Wskip"], dtype=np.float32)
    bq = np.asarray(inputs["bq"], dtype=np.float32)
    # biases are all zeros in this model family (bq folds into q host-side)
    for bn in ("bk", "bv", "bskip"):
        bv = np.asarray(inputs[bn])
        assert np.abs(bv).max() == 0.0, f"nonzero bias {bn} unsupported"

    bf = ml_dtypes.bfloat16
    cT = cen.reshape(B, D).T                                      # [D, B] f32

    # host-side q projection + scaling + fold into per-graph weight columns
    qs = (Wq.T @ cT + bq[:, None]) * (1.0 / np.sqrt(C))           # [F, B]
    qs4 = qs.reshape(H, C, B)
    wkq = np.matmul(Wk.reshape(D, H, C).transpose(1, 0, 2), qs4)  # [H, D, B]
    weq = np.matmul(We.reshape(D, H, C).transpose(1, 0, 2), qs4)
    wkq = np.ascontiguousarray(wkq.transpose(1, 2, 0)).astype(bf)  # [D, B, H]
    weq = np.ascontiguousarray(weq.transpose(1, 2, 0)).astype(bf)

    wvs = np.concatenate([Wv, Ws], axis=1).astype(bf)             # [D, 512]
    web = We.astype(bf)
    ctb = cT.astype(bf)
    ones = np.ones((128, 1), dtype=np.float32).astype(bf)

    # [D, B, x|e] combined layout: one 2D DMA per (group, d-chunk)
    xe = np.empty((D, B, 2 * N), dtype=bf)
    xe[:, :, 0:N] = x.transpose(2, 0, 1)
    xe[:, :, N:2 * N] = e.transpose(2, 0, 1)

    if "nc" not in _cached:
        _cached["nc"] = _build_nc()
    nc = _cached["nc"]

    in_maps = []
    for c in range(NCORES):
        gsl = slice(c * BG, (c + 1) * BG)
        in_maps.append({
            "xe": np.ascontiguousarray(xe[:, gsl]),
            "wkq": np.ascontiguousarray(wkq[:, gsl]),
            "weq": np.ascontiguousarray(weq[:, gsl]),
            "wvs": wvs, "we": web,
            "ct": np.ascontiguousarray(ctb[:, gsl]),
            "ones": ones,
        })

    from concourse.bass_utils import run_bass_kernel_spmd
    res = run_bass_kernel_spmd(nc, in_maps, core_ids=list(range(NCORES)),
                               **_cached.get("run_kwargs", {}))
    _cached["last_results"] = res

    out = np.empty((B, ROWS, F), dtype=np.float32)
    for c, r in enumerate(res.results):
        gsl = slice(c * BG, (c + 1) * BG)
        skip = np.asarray(r["skip"]).astype(np.float32)       # [N, BG, F]
        out[gsl, 1:ROWS, :] = skip.transpose(1, 0, 2)
        aggT = np.asarray(r["aggT"])                          # [128, 2, BG]
        skT = np.asarray(r["skT"])
        s = np.asarray(r["sums"]).reshape(BG, H)              # [BG, H]
        agg = aggT.transpose(2, 1, 0).reshape(BG, F)          # [BG, 256]
        skc = skT.transpose(2, 1, 0).reshape(BG, F)
        out[gsl, 0, :] = skc + agg / np.repeat(s, C, axis=1)
    return out.reshape(B * ROWS, F)


# revision 24
# speedup vs baseline: 1.0347x; 1.0347x over previous
"""Trainium2 Bass kernel for GraphTransformerNet (star-graph TransformerConv).

Shapes (hardcoded): B=1024 graphs, N=128 neighbors, D=256 in-dim,
H=4 heads x C=64 = F=256 out-dim. Data-parallel over 8 NeuronCores
(128 graphs/core).

Key structure (v2 — scores via host-folded q):
  The attention logits only need q.k = x @ (Wk q) + e @ (We q), so the
  per-graph q vector is folded into tiny per-graph weight columns
  wkq[d,g,h] host-side. That removes the whole kT projection pipeline
  (a third of PE streaming) and its PSUM->SBUF casts. Per graph the PE
  runs 4 big matmuls ([v|skip] from x, v from e; stationary = the
  graph's x/e d-chunk) plus 4 nearly-free 4-column score matmuls that
  reuse those stationaries.

  Softmax is max-free (scores ~ N(0,2), no overflow): Act exps the
  [128n, 32(g,h)] score block; the sums over n come from a ones-vector
  matmul; normalization happens on the host (agg and sums ship raw).
  Aggregation = 4 ap=1 matmuls per graph into a persistent PSUM tile
  [128 f_loc, 2 fc, BG], consumed one group behind the producer so the
  PE never waits on Act. Central skip projection runs once at the end
  into its own PSUM; host adds agg/sums to it in f32.

  Inputs ship as [D, BG, x|e] so each group needs just two 2D DMAs;
  skip rows buffer per-group and leave in one DMA as bf16 to an
  [N, BG, F] layout (host transposes back).
"""

import sys

import numpy as np

for _p in ("/opt/trn_rl_repo",):
    if _p not in sys.path:
        sys.path.insert(0, _p)

import ml_dtypes

import concourse.bacc as bacc
import concourse.bass as bass
import concourse.mybir as mybir
from concourse.bass import MemorySpace
from concourse.tile import TileContext

BF16 = mybir.dt.bfloat16
F32 = mybir.dt.float32
AFT = mybir.ActivationFunctionType

B, N, D, H, C = 1024, 128, 256, 4, 64
F = H * C            # 256
NCORES = 8
BG = B // NCORES     # 128 graphs per core
GROUP = 8            # graphs per group (softmax/DMA batch)
NG = BG // GROUP     # 16 groups
ROWS = N + 1         # 129 output rows per graph

_cached = {}


def _build_nc():
    nc = bacc.Bacc()

    xe_d = nc.dram_tensor("xe", [D, BG, 2 * N], BF16, kind="ExternalInput")
    wkq_d = nc.dram_tensor("wkq", [D, BG, H], BF16, kind="ExternalInput")
    weq_d = nc.dram_tensor("weq", [D, BG, H], BF16, kind="ExternalInput")
    wvs_d = nc.dram_tensor("wvs", [D, 2 * F], BF16, kind="ExternalInput")
    we_d = nc.dram_tensor("we", [D, F], BF16, kind="ExternalInput")
    ct_d = nc.dram_tensor("ct", [D, BG], BF16, kind="ExternalInput")
    ones_d = nc.dram_tensor("ones", [128, 1], BF16, kind="ExternalInput")

    skip_d = nc.dram_tensor("skip", [N, BG, F], BF16, kind="ExternalOutput")
    aggT_d = nc.dram_tensor("aggT", [128, 2, BG], F32, kind="ExternalOutput")
    skT_d = nc.dram_tensor("skT", [128, 2, BG], F32, kind="ExternalOutput")
    sums_d = nc.dram_tensor("sums", [1, BG * H], F32, kind="ExternalOutput")

    with TileContext(nc) as tc:
        with (
            tc.tile_pool(name="consts", bufs=1) as consts,
            tc.tile_pool(name="io", bufs=8) as io,
            tc.tile_pool(name="vsb", bufs=2 * GROUP + 2) as v_pool,
            tc.tile_pool(name="skipsb", bufs=2) as skip_pool,
            tc.tile_pool(name="expsb", bufs=3) as exp_pool,
            tc.tile_pool(name="scsum", bufs=2) as scsum_pool,
            tc.tile_pool(name="misc", bufs=6) as misc,
            tc.tile_pool(name="vs_ps", bufs=3, space=MemorySpace.PSUM) as vs_psp,
            tc.tile_pool(name="sc_ps", bufs=2, space=MemorySpace.PSUM) as sc_psp,
            tc.tile_pool(name="agg_ps", bufs=1, space=MemorySpace.PSUM) as agg_psp,
            tc.tile_pool(name="sum_ps", bufs=1, space=MemorySpace.PSUM) as sum_psp,
            tc.tile_pool(name="skc_ps", bufs=1, space=MemorySpace.PSUM) as skc_psp,
        ):
            # ---- constants (spread across trigger queues for a fast ramp) ----
            wvs_sb, we_sb, wkq_sb, weq_sb, ct_sb = [], [], [], [], []
            for dc in range(2):
                dsl = slice(dc * 128, (dc + 1) * 128)
                t = consts.tile([128, 2 * F], BF16, tag=f"wvs{dc}")
                nc.sync.dma_start(t[:, :], wvs_d[dsl, :]); wvs_sb.append(t)
                t = consts.tile([128, F], BF16, tag=f"we{dc}")
                nc.sync.dma_start(t[:, :], we_d[dsl, :]); we_sb.append(t)
                t = consts.tile([128, BG, H], BF16, tag=f"wkq{dc}")
                nc.gpsimd.dma_start(t[:, :, :], wkq_d[dsl, :, :]); wkq_sb.append(t)
                t = consts.tile([128, BG, H], BF16, tag=f"weq{dc}")
                nc.scalar.dma_start(t[:, :, :], weq_d[dsl, :, :]); weq_sb.append(t)
                t = consts.tile([128, BG], BF16, tag=f"ct{dc}")
                nc.scalar.dma_start(t[:, :], ct_d[dsl, :]); ct_sb.append(t)
            ones_sb = consts.tile([128, 1], BF16, tag="ones")
            nc.gpsimd.dma_start(ones_sb[:, :], ones_d[:, :])

            # persistent PSUM: unnormalized aggregated messages in
            # [f_loc, fc, g] layout, and per-(g,h) exp-sums. Each element
            # is written by exactly one matmul; start=True only on the
            # very first write into each bank (clears has_written for the
            # whole bank -> first touch of every address overwrites).
            agg_ps = agg_psp.tile([128, 2, BG], F32, tag="agg")
            sums_ps = sum_psp.tile([1, BG * H], F32, tag="sums")

            # central skip projection runs up front (only needs consts) so
            # the tail after the last group stays short
            skc_ps = skc_psp.tile([128, 2, BG], F32, tag="skc")
            for fc in range(2):
                fsl = slice(F + fc * 128, F + (fc + 1) * 128)
                for dc in range(2):
                    nc.tensor.matmul(skc_ps[:, fc, :], wvs_sb[dc][:, fsl],
                                     ct_sb[dc][:, :],
                                     start=(fc == 0 and dc == 0),
                                     stop=(fc == 1 and dc == 1),
                                     skip_group_check=(fc == 1))
            skT_sb = misc.tile([128, 2, BG], F32, tag="skT")
            nc.scalar.activation(skT_sb[:, :, :], skc_ps[:, :, :], AFT.Copy)
            nc.sync.dma_start(skT_d[:, :, :], skT_sb[:, :, :])

            state = {}

            def consume(j):
                exp_sb, v_sbs = state.pop(j)
                nc.tensor.matmul(sums_ps[0:1, j * 32:(j + 1) * 32],
                                 ones_sb[:, :], exp_sb[:, :],
                                 start=(j == 0), stop=(j == NG - 1),
                                 skip_group_check=(j > 0))
                for gg in range(GROUP):
                    g = j * GROUP + gg
                    for fc in range(2):
                        for hh in range(2):
                            h = fc * 2 + hh
                            first = (g == 0 and fc == 0 and hh == 0)
                            last = (g == BG - 1 and fc == 1 and hh == 1)
                            nc.tensor.matmul(
                                agg_ps[hh * 64:(hh + 1) * 64, fc, g:g + 1],
                                v_sbs[gg][:, fc * 128 + hh * 64:
                                          fc * 128 + (hh + 1) * 64],
                                exp_sb[:, gg * 4 + h:gg * 4 + h + 1],
                                start=first, stop=last,
                                skip_group_check=not first)

            # ---- main loop over groups of 8 graphs ----
            for grp in range(NG):
                g0 = grp * GROUP
                xe = []
                for dc in range(2):
                    dsl = slice(dc * 128, (dc + 1) * 128)
                    t = io.tile([128, GROUP, 2 * N], BF16, tag=f"xe{dc}")
                    if grp == 0:
                        # first tiles gate the PE ramp: split each across two
                        # trigger queues so the four halves transfer at once
                        h = GROUP // 2
                        eng0 = nc.gpsimd if dc == 0 else nc.gpsimd
                        eng1 = nc.sync if dc == 0 else nc.scalar
                        eng0.dma_start(t[:, 0:h, :], xe_d[dsl, g0:g0 + h, :])
                        eng1.dma_start(t[:, h:GROUP, :],
                                       xe_d[dsl, g0 + h:g0 + GROUP, :])
                    else:
                        # steady state: the otherwise-idle GpSimd queue (the
                        # sync queue head-of-line-blocks behind skip-out DMAs)
                        nc.gpsimd.dma_start(t[:, :, :],
                                            xe_d[dsl, g0:g0 + GROUP, :])
                    xe.append(t)

                # four score regions, one per contraction chunk, every write
                # an overwrite: a tiny accumulating matmul pays the PSUM
                # read-modify-write latency (~150-290ns) that can't pipeline
                # over 4 addresses, while tiny overwrites cost ~19ns. The
                # regions are summed on DVE instead.
                sc_ps = sc_psp.tile([128, 4, GROUP * H], F32, tag="scps")
                skip_t = skip_pool.tile([128, GROUP, F], BF16, tag="skipsb")
                v_sbs = []

                for gg in range(GROUP):
                    g = g0 + gg
                    ssl = slice(gg * 4, gg * 4 + 4)
                    xs = [xe[0][:, gg, 0:N], xe[1][:, gg, 0:N]]
                    es = [xe[0][:, gg, N:2 * N], xe[1][:, gg, N:2 * N]]
                    vs_ps = vs_psp.tile([128, 2 * F], F32, tag="vsps")
                    # each score matmul directly follows a vs matmul with the
                    # IDENTICAL stationary: ldweights=False makes it reuse the
                    # already-loaded PE array instead of a ~100ns reload
                    nc.tensor.matmul(vs_ps[:, :], xs[0], wvs_sb[0][:, :],
                                     start=True, stop=False)
                    nc.tensor.matmul(sc_ps[:, 0, ssl], xs[0],
                                     wkq_sb[0][:, g, :],
                                     start=(gg == 0), stop=False,
                                     skip_group_check=(gg > 0)
                                     ).ins.ldweights = False
                    nc.tensor.matmul(vs_ps[:, :], xs[1], wvs_sb[1][:, :],
                                     start=False, stop=False)
                    nc.tensor.matmul(sc_ps[:, 1, ssl], xs[1],
                                     wkq_sb[1][:, g, :],
                                     start=False, stop=False,
                                     skip_group_check=True
                                     ).ins.ldweights = False
                    nc.tensor.matmul(vs_ps[:, 0:F], es[0], we_sb[0][:, :],
                                     start=False, stop=False,
                                     skip_group_check=True)
                    nc.tensor.matmul(sc_ps[:, 2, ssl], es[0],
                                     weq_sb[0][:, g, :],
                                     start=False, stop=False,
                                     skip_group_check=True
                                     ).ins.ldweights = False
                    nc.tensor.matmul(vs_ps[:, 0:F], es[1], we_sb[1][:, :],
                                     start=False, stop=True,
                                     skip_group_check=True)
                    nc.tensor.matmul(sc_ps[:, 3, ssl], es[1],
                                     weq_sb[1][:, g, :],
                                     start=False, stop=(gg == GROUP - 1),
                                     skip_group_check=True
                                     ).ins.ldweights = False

                    v_sb = v_pool.tile([128, F], BF16, tag="vsb")
                    nc.vector.tensor_copy(v_sb[:, :], vs_ps[:, 0:F])
                    v_sbs.append(v_sb)
                    nc.scalar.activation(skip_t[:, gg, :], vs_ps[:, F:2 * F],
                                         AFT.Copy)
                nc.sync.dma_start(skip_d[:, g0:g0 + GROUP, :], skip_t[:, :, :])

                # combine the four regions on DVE (at most one PSUM operand
                # per op), then a single exp on Act
                sc_t = scsum_pool.tile([128, 3, GROUP * H], F32, tag="scsum")
                nc.vector.tensor_copy(sc_t[:, 0, :], sc_ps[:, 1, :])
                nc.vector.tensor_add(sc_t[:, 1, :], sc_ps[:, 0, :],
                                     sc_t[:, 0, :])
                nc.vector.tensor_add(sc_t[:, 0, :], sc_ps[:, 2, :],
                                     sc_t[:, 1, :])
                nc.vector.tensor_add(sc_t[:, 2, :], sc_ps[:, 3, :],
                                     sc_t[:, 0, :])
                exp_sb = exp_pool.tile([128, GROUP * H], BF16, tag="expsb")
                nc.scalar.activation(exp_sb[:, :], sc_t[:, 2, :], AFT.Exp)
                state[grp] = (exp_sb, v_sbs)
                if grp >= 1:
                    consume(grp - 1)

            consume(NG - 1)

            # ---- ship raw agg / sums; host normalizes ----
            aggT_sb = misc.tile([128, 2, BG], F32, tag="aggT")
            nc.vector.tensor_copy(aggT_sb[:, :, :], agg_ps[:, :, :])
            nc.sync.dma_start(aggT_d[:, :, :], aggT_sb[:, :, :])
            sums_sb = misc.tile([1, BG * H], F32, tag="sumsb")
            nc.vector.tensor_copy(sums_sb[:, :], sums_ps[:, :])
            nc.sync.dma_start(sums_d[:, :], sums_sb[:, :])

    nc.compile()
    return nc


def kernel(**inputs):
    x = np.asarray(inputs["neighbor_node_features"], dtype=np.float32)   # [B, N, D]
    e = np.asarray(inputs["edge_features"], dtype=np.float32)            # [B, N, D]
    cen = np.asarray(inputs["central_node_features"], dtype=np.float32)  # [B, 1, D]
    Wq = np.asarray(inputs["Wq"], dtype=np.float32)
    Wk = np.asarray(inputs["Wk"], dtype=np.float32)
    Wv = np.asarray(inputs["Wv"], dtype=np.float32)
    We = np.asarray(inputs["We"], dtype=np.float32)
    Ws = np.asarray(inputs["Wskip"], dtype=np.float32)
    bq = np.asarray(inputs["bq"], dtype=np.float32)
    # biases are all zeros in this model family (bq folds into q host-side)
    for bn in ("bk", "bv", "bskip"):
        bv = np.asarray(inputs[bn])
        assert np.abs(bv).max() == 0.0, f"nonzero bias {bn} unsupported"

    bf = ml_dtypes.bfloat16
    cT = cen.reshape(B, D).T                                      # [D, B] f32

    # host-side q projection + scaling + fold into per-graph weight columns
    qs = (Wq.T @ cT + bq[:, None]) * (1.0 / np.sqrt(C))           # [F, B]
    qs4 = qs.reshape(H, C, B)
    wkq = np.matmul(Wk.reshape(D, H, C).transpose(1, 0, 2), qs4)  # [H, D, B]
    weq = np.matmul(We.reshape(D, H, C).transpose(1, 0, 2), qs4)
    wkq = np.ascontiguousarray(wkq.transpose(1, 2, 0)).astype(bf)  # [D, B, H]
    weq = np.ascontiguousarray(weq.transpose(1, 2, 0)).astype(bf)

    wvs = np.concatenate([Wv, Ws], axis=1).astype(bf)             # [D, 512]
    web = We.astype(bf)
    ctb = cT.astype(bf)
    ones = np.ones((128, 1), dtype=np.float32).astype(bf)

    # [D, B, x|e] combined layout: one 2D DMA per (group, d-chunk)
    xe = np.empty((D, B, 2 * N), dtype=bf)
    xe[:, :, 0:N] = x.transpose(2, 0, 1)
    xe[:, :, N:2 * N] = e.transpose(2, 0, 1)

    if "nc" not in _cached:
        _cached["nc"] = _build_nc()
    nc = _cached["nc"]

    in_maps = []
    for c in range(NCORES):
        gsl = slice(c * BG, (c + 1) * BG)
        in_maps.append({
            "xe": np.ascontiguousarray(xe[:, gsl]),
            "wkq": np.ascontiguousarray(wkq[:, gsl]),
            "weq": np.ascontiguousarray(weq[:, gsl]),
            "wvs": wvs, "we": web,
            "ct": np.ascontiguousarray(ctb[:, gsl]),
            "ones": ones,
        })

    from concourse.bass_utils import run_bass_kernel_spmd
    res = run_bass_kernel_spmd(nc, in_maps, core_ids=list(range(NCORES)),
                               **_cached.get("run_kwargs", {}))
    _cached["last_results"] = res

    out = np.empty((B, ROWS, F), dtype=np.float32)
    for c, r in enumerate(res.results):
        gsl = slice(c * BG, (c + 1) * BG)
        skip = np.asarray(r["skip"]).astype(np.float32)       # [N, BG, F]
        out[gsl, 1:ROWS, :] = skip.transpose(1, 0, 2)
        aggT = np.asarray(r["aggT"])                          # [128, 2, BG]
        skT = np.asarray(r["skT"])
        s = np.asarray(r["sums"]).reshape(BG, H)              # [BG, H]
        agg = aggT.transpose(2, 1, 0).reshape(BG, F)          # [BG, 256]
        skc = skT.transpose(2, 1, 0).reshape(BG, F)
        out[gsl, 0, :] = skc + agg / np.repeat(s, C, axis=1)
    return out.reshape(B * ROWS, F)
